# revision 9
# baseline (speedup 1.0000x reference)
"""Trainium2 Bass kernel for the AxialShift block (4x96x256x256, fp32).

Self-contained: builds an 8-core SPMD Bass program, compiles it once,
and runs it via run_bass_kernel_spmd.

Sharding: each core runs S=2 independent streams; stream s of core k
handles a quarter-sample slab (64 rows) of sample 2s + k//4. The two
streams are interleaved in emission order so that each stream's
GroupNorm AllReduce latency hides under the other stream's compute.

Per-stream pipeline (x read once as fp16, out written once):
  phase A : conv1 (fp16 matmul, M padded to 128 for fast weight load)
            over the 66-row frame (64 true rows + 1 halo row each side);
            stored fp16 into a zero-padded [96, 66, 258] frame;
            per-channel bn_stats partials on the true region.
  AR1     : 8-byte AllReduce of GroupNorm-1 partials over the 4 cores
            sharing the sample.
  GN+B    : fused per-channel scale/bias + erf-Gelu in place (strided,
            interleaved with phase B so branch matmuls start early);
            the 4 axial-shift branch convs as 12 chunk-masked K=96
            matmuls per 512-pixel tile (shifts are free-dim offsets
            into the padded frame), fused bias+Gelu on ScalarE, branch
            sum on VectorE, bn_stats partials for GroupNorm-2.
  AR2     : second 8-byte AllReduce.
  phase C : GroupNorm-2 folded into conv3 (weights scaled by gamma2/std
            on device; mean/bias folded into a per-channel bias applied
            during PSUM->SBUF evacuation, alternating ScalarE/VectorE).
"""
import sys

sys.path.insert(0, "/opt/trn_rl_repo")

import numpy as np

import concourse.bass as bass
import concourse.bacc as bacc
import concourse.tile as tile
from concourse import mybir

F32 = mybir.dt.float32
F16 = mybir.dt.float16

C = 96
M = 128           # matmul output width (96 channels + 32 zero pad -> FWL)
H = 256
W = 256
B = 4
WP = W + 2
N_CORES = 8
S = 2             # streams per core
ROWS_SLAB = H * B // (N_CORES * S)       # 64
EPS = 1e-5
AF = mybir.ActivationFunctionType
ALU = mybir.AluOpType

# (dh, dw) read offsets per chunk j=0,1,2 (s_j = -1, 0, +1):
BR_LR = [(0, 1), (0, 0), (0, -1)]
BR_LDIAG = [(1, 1), (0, 0), (-1, -1)]
BR_TD = [(1, 0), (0, 0), (-1, 0)]
BR_RDIAG = [(1, -1), (0, 0), (-1, 1)]
BRANCHES = [(0, BR_LR), (0, BR_LDIAG), (3, BR_TD), (3, BR_RDIAG)]


def _bcast(ap, nparts):
    return bass.AP(tensor=ap.tensor, offset=ap.offset,
                   ap=[[0, nparts]] + list(ap.ap[1:]))


class _Stream:
    """Per-stream state; stages are emitted by the orchestrator."""

    def __init__(self, nc, tc, pools, RH, groups, io, s):
        self.nc, self.tc, self.s = nc, tc, s
        self.p = pools
        self.RH = RH
        self.RF = RH + 2
        self.NT = RH // 2
        self.NPT = self.RF // 2
        self.groups = groups
        self.inv_n = 1.0 / (len(groups[0]) * C)
        self.io = io
        con = pools["consts"]
        big = pools["big"]
        self.xact = big.tile([C, self.RF, WP], F16, name=f"xact{s}")
        self.opre = big.tile([C, self.RH * W], F16, name=f"opre{s}")
        self.stats1 = con.tile([C, self.NPT, 6], F32, name=f"st1_{s}")
        self.stats2 = con.tile([C, self.NT, 6], F32, name=f"st2_{s}")
        dram = pools["dram"]
        self.d1i = dram.tile([1, 2], F32, name=f"d1i{s}")
        self.d1o = dram.tile([1, 2], F32, name=f"d1o{s}")
        self.d2i = dram.tile([1, 2], F32, name=f"d2i{s}")
        self.d2o = dram.tile([1, 2], F32, name=f"d2o{s}")

    # ---------------- phase A ----------------
    def stage_a(self, psa):
        nc, s = self.nc, self.s
        con = self.p["consts"]
        xin = self.p["xin"]
        nc.vector.memset(self.xact[:, :, 0:1], 0.0)
        nc.vector.memset(self.xact[:, :, WP - 1:WP], 0.0)
        xs = self.io["xs"][s]
        xt, xt_base = None, 0
        for b0 in range(0, self.NPT, 2):
            nb = min(2, self.NPT - b0)
            r0 = 2 * b0
            if r0 % 8 == 0:
                xt = xin.tile([C, 8, W], F16, tag="xt")
                nrows = min(8, self.RF - r0)
                nc.sync.dma_start(out=xt[:, 0:nrows, :],
                                  in_=xs[:, r0:r0 + nrows, :])
                xt_base = r0
            pt = psa.tile([M, 2, 512], F32, tag="pta")
            for j in range(nb):
                rr = r0 + 2 * j - xt_base
                nc.tensor.matmul(out=pt[:, j, :], lhsT=self.p["w1t"][:],
                                 rhs=xt[:, rr:rr + 2, :],
                                 start=True, stop=True)
            cp_out = self.xact[:, r0:r0 + 2 * nb, 1:W + 1].rearrange(
                "p (n r) w -> p n r w", r=2)
            cp_in = pt[0:C, 0:nb, :].rearrange("p n (r w) -> p n r w", w=W)
            if (b0 // 2) % 2 == 0:
                nc.scalar.copy(out=cp_out, in_=cp_in)
            else:
                nc.vector.tensor_copy(out=cp_out, in_=cp_in)
            for j in range(nb):
                ti = b0 + j
                if ti == 0:
                    src = pt[0:C, j, 256:512]
                elif ti == self.NPT - 1:
                    src = pt[0:C, j, 0:256]
                else:
                    src = pt[0:C, j, :]
                nc.vector.bn_stats(out=self.stats1[:, ti, :], in_=src)

        # GN1 partials -> AllReduce input
        cols = self.p["cols"]
        mv1 = con.tile([C, 2], F32, name=f"mv1_{s}")
        nc.vector.bn_aggr(out=mv1[:], in_=self.stats1[:])
        pack1 = con.tile([C, 2], F32, name=f"pk1_{s}")
        nc.vector.tensor_add(out=pack1[:, 0:1], in0=mv1[:, 0:1],
                             in1=cols[:, 0:1])
        tsq = con.tile([C, 1], F32, name=f"t1q_{s}")
        nc.vector.tensor_mul(out=tsq[:], in0=pack1[:, 0:1], in1=pack1[:, 0:1])
        nc.vector.tensor_add(out=pack1[:, 1:2], in0=mv1[:, 1:2], in1=tsq[:])
        spt = psa.tile([M, 2, 512], F32, tag="pta")
        nc.tensor.matmul(out=spt[0:1, 0, 0:2], lhsT=self.p["ones96"][:],
                         rhs=pack1[:], start=True, stop=True)
        ar_in = con.tile([1, 2], F32, name=f"ar1i_{s}")
        nc.scalar.copy(out=ar_in[:], in_=spt[0:1, 0, 0:2])
        nc.sync.dma_start(out=self.d1i[:], in_=ar_in[:])
        nc.gpsimd.collective_compute(
            "AllReduce", ALU.add, replica_groups=self.groups,
            ins=[self.d1i.opt()], outs=[self.d1o.opt()])

    # ---------------- GN1 scalars ----------------
    def post_ar1(self):
        nc, s = self.nc, self.s
        con = self.p["consts"]
        cols = self.p["cols"]
        ar1 = con.tile([C, 2], F32, name=f"ar1_{s}")
        nc.gpsimd.dma_start(out=ar1[:], in_=_bcast(self.d1o[:], C))
        mu = con.tile([C, 1], F32, name=f"mu1_{s}")
        nc.vector.tensor_scalar_mul(out=mu[:], in0=ar1[:, 0:1],
                                    scalar1=self.inv_n)
        var = con.tile([C, 1], F32, name=f"v1_{s}")
        nc.vector.tensor_scalar_mul(out=var[:], in0=ar1[:, 1:2],
                                    scalar1=self.inv_n)
        musq = con.tile([C, 1], F32, name=f"mq1_{s}")
        nc.vector.tensor_mul(out=musq[:], in0=mu[:], in1=mu[:])
        nc.vector.tensor_sub(out=var[:], in0=var[:], in1=musq[:])
        std = con.tile([C, 1], F32, name=f"sd1_{s}")
        nc.scalar.activation(out=std[:], in_=var[:], func=AF.Sqrt,
                             bias=self.p["epsb"][:])
        inv = con.tile([C, 1], F32, name=f"iv1_{s}")
        nc.vector.reciprocal(out=inv[:], in_=std[:])
        self.scale1 = con.tile([C, 1], F32, name=f"sc1_{s}")
        nc.vector.tensor_mul(out=self.scale1[:], in0=inv[:], in1=cols[:, 1:2])
        self.bias1 = con.tile([C, 1], F32, name=f"bi1_{s}")
        nc.vector.tensor_sub(out=self.bias1[:], in0=cols[:, 0:1], in1=mu[:])
        nc.vector.tensor_mul(out=self.bias1[:], in0=self.bias1[:],
                             in1=self.scale1[:])
        nc.vector.tensor_add(out=self.bias1[:], in0=self.bias1[:],
                             in1=cols[:, 2:3])

    def _gn_chunk(self, r0, r1):
        nc = self.nc
        nc.scalar.activation(out=self.xact[:, r0:r1, 1:W + 1],
                             in_=self.xact[:, r0:r1, 1:W + 1],
                             func=AF.Gelu, bias=self.bias1[:],
                             scale=self.scale1[:])
        if r0 == 0:
            nc.vector.tensor_scalar_mul(out=self.xact[:, 0:1, :],
                                        in0=self.xact[:, 0:1, :],
                                        scalar1=self.p["em"][:, 2 * self.s:
                                                            2 * self.s + 1])
        if r1 == self.RF:
            nc.vector.tensor_scalar_mul(
                out=self.xact[:, self.RF - 1:self.RF, :],
                in0=self.xact[:, self.RF - 1:self.RF, :],
                scalar1=self.p["em"][:, 2 * self.s + 1:2 * self.s + 2])

    # ---------------- GN-apply + phase B ----------------
    def stage_b(self, psb):
        nc, s = self.nc, self.s
        con = self.p["consts"]
        cols = self.p["cols"]
        wbm = self.p["wbm"]
        gst, tmp = self.p["gst"], self.p["tmp"]
        gn_r = 0
        for t in range(self.NT):
            need = min(2 * t + 4, self.RF)
            while gn_r < need:
                r1 = min(gn_r + 8, self.RF)
                self._gn_chunk(gn_r, r1)
                gn_r = r1
            pr = 2 * t + 1
            pt = psb.tile([M, 4, 512], F32, tag="ptb")
            for b, (wsel, ds) in enumerate(BRANCHES):
                for j, (dh, dw) in enumerate(ds):
                    bi = wsel + j
                    nc.tensor.matmul(
                        out=pt[:, b, :],
                        lhsT=wbm[:, bi * M:(bi + 1) * M],
                        rhs=self.xact[0:C, pr + dh:pr + dh + 2,
                                      1 + dw:1 + dw + W],
                        start=(j == 0), stop=(j == 2))
            g = gst.tile([C, 4, 512], F16, tag="g")
            nc.scalar.activation(out=g[:, 0:2, :], in_=pt[0:C, 0:2, :],
                                 func=AF.Gelu, bias=cols[:, 3:4])
            nc.scalar.activation(out=g[:, 2:4, :], in_=pt[0:C, 2:4, :],
                                 func=AF.Gelu, bias=cols[:, 4:5])
            o1 = tmp.tile([C, 512], F16, tag="o1")
            o2 = tmp.tile([C, 512], F16, tag="o2")
            nc.vector.tensor_add(out=o1[:], in0=g[:, 0, :], in1=g[:, 1, :])
            nc.vector.tensor_add(out=o2[:], in0=g[:, 2, :], in1=g[:, 3, :])
            od = self.opre[:, 512 * t:512 * (t + 1)]
            nc.vector.tensor_add(out=od, in0=o1[:], in1=o2[:])
            nc.vector.bn_stats(out=self.stats2[:, t, :], in_=od)

        mv2 = con.tile([C, 2], F32, name=f"mv2_{s}")
        nc.vector.bn_aggr(out=mv2[:], in_=self.stats2[:])
        pack2 = con.tile([C, 2], F32, name=f"pk2_{s}")
        nc.vector.tensor_copy(out=pack2[:, 0:1], in_=mv2[:, 0:1])
        tsq = con.tile([C, 1], F32, name=f"t2q_{s}")
        nc.vector.tensor_mul(out=tsq[:], in0=mv2[:, 0:1], in1=mv2[:, 0:1])
        nc.vector.tensor_add(out=pack2[:, 1:2], in0=mv2[:, 1:2], in1=tsq[:])
        spt = psb.tile([M, 4, 512], F32, tag="ptb")
        nc.tensor.matmul(out=spt[0:1, 0, 0:2], lhsT=self.p["ones96"][:],
                         rhs=pack2[:], start=True, stop=True)
        ar_in = con.tile([1, 2], F32, name=f"ar2i_{s}")
        nc.scalar.copy(out=ar_in[:], in_=spt[0:1, 0, 0:2])
        nc.sync.dma_start(out=self.d2i[:], in_=ar_in[:])
        nc.gpsimd.collective_compute(
            "AllReduce", ALU.add, replica_groups=self.groups,
            ins=[self.d2i.opt()], outs=[self.d2o.opt()])

    # ---------------- GN2 scalars + conv3 weight fold ----------------
    def post_ar2(self):
        nc, s = self.nc, self.s
        con = self.p["consts"]
        cols = self.p["cols"]
        ar2 = con.tile([C, 2], F32, name=f"ar2_{s}")
        nc.gpsimd.dma_start(out=ar2[:], in_=_bcast(self.d2o[:], C))
        mu = con.tile([C, 1], F32, name=f"mu2_{s}")
        nc.vector.tensor_scalar_mul(out=mu[:], in0=ar2[:, 0:1],
                                    scalar1=self.inv_n)
        var = con.tile([C, 1], F32, name=f"v2_{s}")
        nc.vector.tensor_scalar_mul(out=var[:], in0=ar2[:, 1:2],
                                    scalar1=self.inv_n)
        musq = con.tile([C, 1], F32, name=f"mq2_{s}")
        nc.vector.tensor_mul(out=musq[:], in0=mu[:], in1=mu[:])
        nc.vector.tensor_sub(out=var[:], in0=var[:], in1=musq[:])
        std = con.tile([C, 1], F32, name=f"sd2_{s}")
        nc.scalar.activation(out=std[:], in_=var[:], func=AF.Sqrt,
                             bias=self.p["epsb"][:])
        inv = con.tile([C, 1], F32, name=f"iv2_{s}")
        nc.vector.reciprocal(out=inv[:], in_=std[:])
        self.w3ts = con.tile([C, M], F16, name=f"w3s_{s}")
        nc.vector.tensor_scalar_mul(out=self.w3ts[:], in0=self.p["w3gt"][:],
                                    scalar1=inv[:])
        s2 = con.tile([C, 1], F32, name=f"s2_{s}")
        nc.vector.tensor_mul(out=s2[:], in0=inv[:], in1=mu[:])
        self.ccol = con.tile([C, 1], F32, name=f"cc_{s}")
        nc.vector.tensor_mul(out=self.ccol[:], in0=s2[:], in1=cols[:, 6:7])
        nc.vector.tensor_sub(out=self.ccol[:], in0=cols[:, 5:6],
                             in1=self.ccol[:])

    # ---------------- phase C ----------------
    def stage_c(self, psc):
        nc, s = self.nc, self.s
        ost = self.p["ost"]
        out = self.io["out"][s]
        for b0 in range(0, self.NT, 2):
            nb = min(2, self.NT - b0)
            pc = psc.tile([M, 2, 512], F32, tag="ptc")
            for j in range(nb):
                tt = b0 + j
                nc.tensor.matmul(out=pc[:, j, :], lhsT=self.w3ts[:],
                                 rhs=self.opre[:, 512 * tt:512 * (tt + 1)],
                                 start=True, stop=True)
            o = ost.tile([C, 2, 512], F32, tag="o")
            if (b0 // 2) % 2 == 0:
                nc.vector.tensor_scalar(out=o[:, 0:nb, :],
                                        in0=pc[0:C, 0:nb, :],
                                        scalar1=self.ccol[:], scalar2=None,
                                        op0=ALU.add)
            else:
                nc.scalar.activation(out=o[:, 0:nb, :], in_=pc[0:C, 0:nb, :],
                                     func=AF.Identity, bias=self.ccol[:])
            nc.sync.dma_start(
                out=out[:, 2 * b0:2 * b0 + 2 * nb, :].rearrange(
                    "p (n r) w -> p n r w", r=2),
                in_=o[:, 0:nb, :].rearrange("p n (r w) -> p n r w", w=W))


def _emit(nc, tc, ctx, RH, groups, io):
    pools = {
        "consts": ctx.enter_context(tc.tile_pool(name="consts", bufs=1)),
        "big": ctx.enter_context(tc.tile_pool(name="big", bufs=1)),
        "xin": ctx.enter_context(tc.tile_pool(name="xin", bufs=3)),
        "gst": ctx.enter_context(tc.tile_pool(name="gst", bufs=3)),
        "tmp": ctx.enter_context(tc.tile_pool(name="tmp", bufs=4)),
        "ost": ctx.enter_context(tc.tile_pool(name="ost", bufs=4)),
        "dram": ctx.enter_context(tc.tile_pool(name="dram", bufs=1,
                                               space="DRAM")),
    }
    con = pools["consts"]
    w1t = con.tile([C, M], F16)
    nc.sync.dma_start(out=w1t[:], in_=io["w1t"][:])
    wbm = con.tile([C, 6 * M], F16)
    nc.sync.dma_start(out=wbm[:], in_=io["wbm"][:])
    w3gt = con.tile([C, M], F32)
    nc.sync.dma_start(out=w3gt[:], in_=io["w3gt"][:])
    cols = con.tile([C, 7], F32)
    nc.sync.dma_start(out=cols[:], in_=io["cols"][:])
    em = con.tile([C, 2 * S], F32)
    nc.gpsimd.dma_start(out=em[:], in_=_bcast(io["em"][:], C))
    ones96 = con.tile([C, 1], F32)
    nc.vector.memset(ones96[:], 1.0)
    epsb = con.tile([C, 1], F32)
    nc.vector.memset(epsb[:], EPS)
    pools.update(w1t=w1t, wbm=wbm, w3gt=w3gt, cols=cols, em=em,
                 ones96=ones96, epsb=epsb)

    # warm up the collectives firmware path (result unused)
    dw_i = pools["dram"].tile([1, 2], F32)
    dw_o = pools["dram"].tile([1, 2], F32)
    warm = con.tile([1, 2], F32)
    nc.vector.memset(warm[:], 0.0)
    nc.sync.dma_start(out=dw_i[:], in_=warm[:])
    nc.gpsimd.collective_compute("AllReduce", ALU.add, replica_groups=groups,
                                 ins=[dw_i.opt()], outs=[dw_o.opt()])

    streams = [_Stream(nc, tc, pools, RH, groups, io, s) for s in range(S)]

    with tc.tile_pool(name="psa", bufs=4, space="PSUM") as psa:
        for st in streams:
            st.stage_a(psa)
    with tc.tile_pool(name="psb", bufs=2, space="PSUM") as psb:
        for st in streams:
            st.post_ar1()
            st.stage_b(psb)
    with tc.tile_pool(name="psc", bufs=4, space="PSUM") as psc:
        for st in streams:
            st.post_ar2()
            st.stage_c(psc)


def build_program(rows_slab=ROWS_SLAB, n_cores=N_CORES, n_samples=B,
                  n_streams=S):
    import contextlib
    cps = n_cores * n_streams // n_samples      # cores per sample
    groups = [list(range(a * cps, (a + 1) * cps))
              for a in range(n_cores // cps)]
    RF = rows_slab + 2
    nc = bacc.Bacc("TRN2", target_bir_lowering=False, debug=False,
                   enable_asserts=False, num_devices=n_cores)
    io = {
        "xs": nc.dram_tensor("xs", [n_streams, C, RF, W], F16,
                             kind="ExternalInput").ap(),
        "em": nc.dram_tensor("em", [1, 2 * n_streams], F32,
                             kind="ExternalInput").ap(),
        "w1t": nc.dram_tensor("w1t", [C, M], F16, kind="ExternalInput").ap(),
        "wbm": nc.dram_tensor("wbm", [C, 6 * M], F16,
                              kind="ExternalInput").ap(),
        "w3gt": nc.dram_tensor("w3gt", [C, M], F32, kind="ExternalInput").ap(),
        "cols": nc.dram_tensor("cols", [C, 7], F32, kind="ExternalInput").ap(),
        "out": nc.dram_tensor("out", [n_streams, C, rows_slab, W], F32,
                              kind="ExternalOutput").ap(),
    }
    with tile.TileContext(nc) as tc:
        with contextlib.ExitStack() as ctx:
            _emit(nc, tc, ctx, rows_slab, groups, io)
    nc.compile()
    return nc


def host_inputs(x, w1, b1, w21, b21, w22, b22, w3, b3,
                gn1_w, gn1_b, gn2_w, gn2_b,
                rows_slab=ROWS_SLAB, n_cores=N_CORES, n_streams=S):
    x = np.asarray(x, np.float32)
    nb_, _, hh, _ = x.shape
    cps = n_cores * n_streams // nb_
    w1 = np.asarray(w1, np.float32)
    w21 = np.asarray(w21, np.float32)
    w22 = np.asarray(w22, np.float32)
    w3 = np.asarray(w3, np.float32)

    w1t = np.zeros((C, M), np.float16)
    w1t[:, 0:C] = w1.T
    wbm = np.zeros((C, 6 * M), np.float16)
    for wi, wmat in enumerate((w21, w22)):
        wt = np.ascontiguousarray(wmat.T).astype(np.float16)
        for j in range(3):
            blk = np.zeros((C, M), np.float16)
            blk[32 * j:32 * j + 32, 0:C] = wt[32 * j:32 * j + 32, :]
            wbm[:, (3 * wi + j) * M:(3 * wi + j + 1) * M] = blk
    w3gt = np.zeros((C, M), np.float32)
    w3gt[:, 0:C] = (w3 * np.asarray(gn2_w)[None, :]).T
    shared = {
        "w1t": w1t,
        "wbm": wbm,
        "w3gt": w3gt,
        "cols": np.ascontiguousarray(np.stack(
            [np.asarray(b1, np.float32), np.asarray(gn1_w, np.float32),
             np.asarray(gn1_b, np.float32), np.asarray(b21, np.float32),
             np.asarray(b22, np.float32),
             (np.asarray(b3) + w3 @ np.asarray(gn2_b)).astype(np.float32),
             (w3 * np.asarray(gn2_w)[None, :]).sum(1).astype(np.float32)],
            axis=1)),
    }
    x16 = x.astype(np.float16)
    in_maps = []
    for k in range(n_cores):
        xs = np.zeros((n_streams, C, rows_slab + 2, W), np.float16)
        em = np.zeros((1, 2 * n_streams), np.float32)
        for s in range(n_streams):
            bidx = s * (nb_ // n_streams) + k // cps
            q = k % cps
            h0 = q * rows_slab
            lo, hi = h0 - 1, h0 + rows_slab + 1
            slo, shi = max(lo, 0), min(hi, hh)
            xs[s, :, slo - lo:slo - lo + (shi - slo), :] = \
                x16[bidx, :, slo:shi, :]
            em[0, 2 * s] = 1.0 if lo >= 0 else 0.0
            em[0, 2 * s + 1] = 1.0 if hi <= hh else 0.0
        in_maps.append({"xs": xs, "em": em, **shared})
    return in_maps


def gather_output(results, rows_slab=ROWS_SLAB, n_cores=N_CORES,
                  n_streams=S, n_samples=B, hh=H):
    cps = n_cores * n_streams // n_samples
    out = np.empty((n_samples, C, hh, W), np.float32)
    for k in range(n_cores):
        for s in range(n_streams):
            bidx = s * (n_samples // n_streams) + k // cps
            q = k % cps
            out[bidx, :, q * rows_slab:(q + 1) * rows_slab, :] = \
                results[k]["out"][s]
    return out


_PROGRAM = None


def kernel(x, w1, b1, w21, b21, w22, b22, w3, b3, gn1_w, gn1_b, gn2_w, gn2_b):
    global _PROGRAM
    from concourse.bass_utils import run_bass_kernel_spmd
    from concourse.bass_interp import get_hw_module
    if _PROGRAM is None:
        nc = build_program()
        nc.m = get_hw_module(nc.m)
        _PROGRAM = nc
    nc = _PROGRAM
    in_maps = host_inputs(x, w1, b1, w21, b21, w22, b22, w3, b3,
                          gn1_w, gn1_b, gn2_w, gn2_b)
    res = run_bass_kernel_spmd(nc, in_maps, core_ids=list(range(N_CORES)))
    return gather_output(res.results)


# revision 10
# speedup vs baseline: 1.0324x; 1.0324x over previous
"""Trainium2 Bass kernel for the AxialShift block (4x96x256x256, fp32).

Self-contained: builds an 8-core SPMD Bass program, compiles it once,
and runs it via run_bass_kernel_spmd.

Sharding: each core runs S=2 independent streams; stream s of core k
handles a quarter-sample slab (64 rows) of sample 2s + k//4. The two
streams are interleaved in emission order so that each stream's
GroupNorm AllReduce latency hides under the other stream's compute.

Per-stream pipeline (x read once as fp16, out written once):
  phase A : conv1 (fp16 matmul, M padded to 128 for fast weight load)
            over the 66-row frame (64 true rows + 1 halo row each side);
            stored fp16 into a zero-padded [96, 66, 258] frame;
            per-channel bn_stats partials on the true region.
  AR1     : 8-byte AllReduce of GroupNorm-1 partials over the 4 cores
            sharing the sample.
  GN+B    : fused per-channel scale/bias + erf-Gelu in place (strided,
            interleaved with phase B so branch matmuls start early);
            the 4 axial-shift branch convs as 12 chunk-masked K=96
            matmuls per 512-pixel tile (shifts are free-dim offsets
            into the padded frame), fused bias+Gelu on ScalarE, branch
            sum on VectorE, bn_stats partials for GroupNorm-2.
  AR2     : second 8-byte AllReduce.
  phase C : GroupNorm-2 folded into conv3 (weights scaled by gamma2/std
            on device; mean/bias folded into a per-channel bias applied
            during PSUM->SBUF evacuation, alternating ScalarE/VectorE).
"""
import sys

sys.path.insert(0, "/opt/trn_rl_repo")

import numpy as np

import concourse.bass as bass
import concourse.bacc as bacc
import concourse.tile as tile
from concourse import mybir

F32 = mybir.dt.float32
F16 = mybir.dt.float16

C = 96
M = 128           # matmul output width (96 channels + 32 zero pad -> FWL)
H = 256
W = 256
B = 4
WP = W + 2
N_CORES = 8
S = 2             # streams per core
ROWS_SLAB = H * B // (N_CORES * S)       # 64
EPS = 1e-5
AF = mybir.ActivationFunctionType
ALU = mybir.AluOpType

# (dh, dw) read offsets per chunk j=0,1,2 (s_j = -1, 0, +1):
BR_LR = [(0, 1), (0, 0), (0, -1)]
BR_LDIAG = [(1, 1), (0, 0), (-1, -1)]
BR_TD = [(1, 0), (0, 0), (-1, 0)]
BR_RDIAG = [(1, -1), (0, 0), (-1, 1)]
BRANCHES = [(0, BR_LR), (0, BR_LDIAG), (3, BR_TD), (3, BR_RDIAG)]


def _bcast(ap, nparts):
    return bass.AP(tensor=ap.tensor, offset=ap.offset,
                   ap=[[0, nparts]] + list(ap.ap[1:]))


class _Stream:
    """Per-stream state; stages are emitted by the orchestrator."""

    def __init__(self, nc, tc, pools, RH, groups, io, s):
        self.nc, self.tc, self.s = nc, tc, s
        self.p = pools
        self.RH = RH
        self.RF = RH + 2
        self.NT = RH // 2
        self.NPT = self.RF // 2
        self.groups = groups
        self.inv_n = 1.0 / (len(groups[0]) * C)
        self.io = io
        con = pools["consts"]
        big = pools["big"]
        self.xact = big.tile([C + 1, self.RF, WP], F16, name=f"xact{s}")
        self.opre = big.tile([C, self.RH * W], F16, name=f"opre{s}")
        self.stats1 = con.tile([C, self.NPT, 6], F32, name=f"st1_{s}")
        self.stats2 = con.tile([C, self.NT, 6], F32, name=f"st2_{s}")
        dram = pools["dram"]
        self.d1i = dram.tile([1, 2], F32, name=f"d1i{s}")
        self.d1o = dram.tile([1, 2], F32, name=f"d1o{s}")
        self.d2i = dram.tile([1, 2], F32, name=f"d2i{s}")
        self.d2o = dram.tile([1, 2], F32, name=f"d2o{s}")

    # ---------------- phase A ----------------
    def stage_a(self, psa):
        nc, s = self.nc, self.s
        con = self.p["consts"]
        xin = self.p["xin"]
        nc.vector.memset(self.xact[0:C, :, 0:1], 0.0)
        nc.vector.memset(self.xact[0:C, :, WP - 1:WP], 0.0)
        onesrow = self.io["onesrow"][:]
        nc.gpsimd.dma_start(
            out=self.xact[C:C + 1, :, :],
            in_=bass.AP(tensor=onesrow.tensor, offset=onesrow.offset,
                        ap=[[0, 1], [0, self.RF]] + list(onesrow.ap[1:])))
        xs = self.io["xs"][s]
        xt, xt_base = None, 0
        for b0 in range(0, self.NPT, 2):
            nb = min(2, self.NPT - b0)
            r0 = 2 * b0
            if r0 % 8 == 0:
                xt = xin.tile([C, 8, W], F16, tag="xt")
                nrows = min(8, self.RF - r0)
                nc.sync.dma_start(out=xt[:, 0:nrows, :],
                                  in_=xs[:, r0:r0 + nrows, :])
                xt_base = r0
            pt = psa.tile([M, 2, 512], F32, tag="pta")
            for j in range(nb):
                rr = r0 + 2 * j - xt_base
                nc.tensor.matmul(out=pt[:, j, :], lhsT=self.p["w1t"][:],
                                 rhs=xt[:, rr:rr + 2, :],
                                 start=True, stop=True)
            cp_out = self.xact[0:C, r0:r0 + 2 * nb, 1:W + 1].rearrange(
                "p (n r) w -> p n r w", r=2)
            cp_in = pt[0:C, 0:nb, :].rearrange("p n (r w) -> p n r w", w=W)
            nc.scalar.copy(out=cp_out, in_=cp_in)
            for j in range(nb):
                ti = b0 + j
                if ti == 0:
                    src = pt[0:C, j, 256:512]
                elif ti == self.NPT - 1:
                    src = pt[0:C, j, 0:256]
                else:
                    src = pt[0:C, j, :]
                nc.vector.bn_stats(out=self.stats1[:, ti, :], in_=src)

        # GN1 partials -> AllReduce input
        cols = self.p["cols"]
        mv1 = con.tile([C, 2], F32, name=f"mv1_{s}")
        nc.vector.bn_aggr(out=mv1[:], in_=self.stats1[:])
        pack1 = con.tile([C, 2], F32, name=f"pk1_{s}")
        nc.vector.tensor_add(out=pack1[:, 0:1], in0=mv1[:, 0:1],
                             in1=cols[:, 0:1])
        tsq = con.tile([C, 1], F32, name=f"t1q_{s}")
        nc.vector.tensor_mul(out=tsq[:], in0=pack1[:, 0:1], in1=pack1[:, 0:1])
        nc.vector.tensor_add(out=pack1[:, 1:2], in0=mv1[:, 1:2], in1=tsq[:])
        spt = psa.tile([M, 2, 512], F32, tag="pta")
        nc.tensor.matmul(out=spt[0:1, 0, 0:2], lhsT=self.p["ones96"][:],
                         rhs=pack1[:], start=True, stop=True)
        ar_in = con.tile([1, 2], F32, name=f"ar1i_{s}")
        nc.scalar.copy(out=ar_in[:], in_=spt[0:1, 0, 0:2])
        nc.sync.dma_start(out=self.d1i[:], in_=ar_in[:])
        nc.gpsimd.collective_compute(
            "AllReduce", ALU.add, replica_groups=self.groups,
            ins=[self.d1i.opt()], outs=[self.d1o.opt()])

    # ---------------- GN1 scalars ----------------
    def post_ar1(self):
        nc, s = self.nc, self.s
        con = self.p["consts"]
        cols = self.p["cols"]
        ar1 = con.tile([C, 2], F32, name=f"ar1_{s}")
        nc.gpsimd.dma_start(out=ar1[:], in_=_bcast(self.d1o[:], C))
        mu = con.tile([C, 1], F32, name=f"mu1_{s}")
        nc.vector.tensor_scalar_mul(out=mu[:], in0=ar1[:, 0:1],
                                    scalar1=self.inv_n)
        var = con.tile([C, 1], F32, name=f"v1_{s}")
        nc.vector.tensor_scalar_mul(out=var[:], in0=ar1[:, 1:2],
                                    scalar1=self.inv_n)
        musq = con.tile([C, 1], F32, name=f"mq1_{s}")
        nc.vector.tensor_mul(out=musq[:], in0=mu[:], in1=mu[:])
        nc.vector.tensor_sub(out=var[:], in0=var[:], in1=musq[:])
        std = con.tile([C, 1], F32, name=f"sd1_{s}")
        nc.scalar.activation(out=std[:], in_=var[:], func=AF.Sqrt,
                             bias=self.p["epsb"][:])
        inv = con.tile([C, 1], F32, name=f"iv1_{s}")
        nc.vector.reciprocal(out=inv[:], in_=std[:])
        self.scale1 = con.tile([C, 1], F32, name=f"sc1_{s}")
        nc.vector.tensor_mul(out=self.scale1[:], in0=inv[:], in1=cols[:, 1:2])
        self.bias1 = con.tile([C, 1], F32, name=f"bi1_{s}")
        nc.vector.tensor_sub(out=self.bias1[:], in0=cols[:, 0:1], in1=mu[:])
        nc.vector.tensor_mul(out=self.bias1[:], in0=self.bias1[:],
                             in1=self.scale1[:])
        nc.vector.tensor_add(out=self.bias1[:], in0=self.bias1[:],
                             in1=cols[:, 2:3])

    def _gn_chunk(self, r0, r1):
        nc = self.nc
        nc.scalar.activation(out=self.xact[0:C, r0:r1, 1:W + 1],
                             in_=self.xact[0:C, r0:r1, 1:W + 1],
                             func=AF.Gelu, bias=self.bias1[:],
                             scale=self.scale1[:])
        if r0 == 0:
            nc.vector.tensor_scalar_mul(out=self.xact[0:C, 0:1, :],
                                        in0=self.xact[0:C, 0:1, :],
                                        scalar1=self.p["em"][:, 2 * self.s:
                                                            2 * self.s + 1])
        if r1 == self.RF:
            nc.vector.tensor_scalar_mul(
                out=self.xact[0:C, self.RF - 1:self.RF, :],
                in0=self.xact[0:C, self.RF - 1:self.RF, :],
                scalar1=self.p["em"][:, 2 * self.s + 1:2 * self.s + 2])

    # ---------------- GN-apply + phase B ----------------
    def stage_b(self, psb):
        nc, s = self.nc, self.s
        con = self.p["consts"]
        cols = self.p["cols"]
        wbm = self.p["wbm"]
        gst, tmp = self.p["gst"], self.p["tmp"]
        gn_r = 0
        for t in range(self.NT):
            need = min(2 * t + 4, self.RF)
            while gn_r < need:
                r1 = min(gn_r + 8, self.RF)
                self._gn_chunk(gn_r, r1)
                gn_r = r1
            pr = 2 * t + 1
            pt = psb.tile([M, 4, 512], F32, tag="ptb")
            for b, (wsel, ds) in enumerate(BRANCHES):
                for j, (dh, dw) in enumerate(ds):
                    bi = wsel + j
                    nc.tensor.matmul(
                        out=pt[:, b, :],
                        lhsT=wbm[:, bi * M:(bi + 1) * M],
                        rhs=self.xact[0:C + 1, pr + dh:pr + dh + 2,
                                      1 + dw:1 + dw + W],
                        start=(j == 0), stop=(j == 2))
            g = gst.tile([C, 4, 512], F16, tag="g")
            nc.scalar.activation(out=g[:], in_=pt[0:C, :, :],
                                 func=AF.Gelu, bias=self.p["zcol"][:])
            o1 = tmp.tile([C, 512], F16, tag="o1")
            o2 = tmp.tile([C, 512], F16, tag="o2")
            nc.vector.tensor_add(out=o1[:], in0=g[:, 0, :], in1=g[:, 1, :])
            nc.vector.tensor_add(out=o2[:], in0=g[:, 2, :], in1=g[:, 3, :])
            od = self.opre[:, 512 * t:512 * (t + 1)]
            nc.vector.tensor_add(out=od, in0=o1[:], in1=o2[:])
            nc.vector.bn_stats(out=self.stats2[:, t, :], in_=od)

        mv2 = con.tile([C, 2], F32, name=f"mv2_{s}")
        nc.vector.bn_aggr(out=mv2[:], in_=self.stats2[:])
        pack2 = con.tile([C, 2], F32, name=f"pk2_{s}")
        nc.vector.tensor_copy(out=pack2[:, 0:1], in_=mv2[:, 0:1])
        tsq = con.tile([C, 1], F32, name=f"t2q_{s}")
        nc.vector.tensor_mul(out=tsq[:], in0=mv2[:, 0:1], in1=mv2[:, 0:1])
        nc.vector.tensor_add(out=pack2[:, 1:2], in0=mv2[:, 1:2], in1=tsq[:])
        spt = psb.tile([M, 4, 512], F32, tag="ptb")
        nc.tensor.matmul(out=spt[0:1, 0, 0:2], lhsT=self.p["ones96"][:],
                         rhs=pack2[:], start=True, stop=True)
        ar_in = con.tile([1, 2], F32, name=f"ar2i_{s}")
        nc.scalar.copy(out=ar_in[:], in_=spt[0:1, 0, 0:2])
        nc.sync.dma_start(out=self.d2i[:], in_=ar_in[:])
        nc.gpsimd.collective_compute(
            "AllReduce", ALU.add, replica_groups=self.groups,
            ins=[self.d2i.opt()], outs=[self.d2o.opt()])

    # ---------------- GN2 scalars + conv3 weight fold ----------------
    def post_ar2(self):
        nc, s = self.nc, self.s
        con = self.p["consts"]
        cols = self.p["cols"]
        ar2 = con.tile([C, 2], F32, name=f"ar2_{s}")
        nc.gpsimd.dma_start(out=ar2[:], in_=_bcast(self.d2o[:], C))
        mu = con.tile([C, 1], F32, name=f"mu2_{s}")
        nc.vector.tensor_scalar_mul(out=mu[:], in0=ar2[:, 0:1],
                                    scalar1=self.inv_n)
        var = con.tile([C, 1], F32, name=f"v2_{s}")
        nc.vector.tensor_scalar_mul(out=var[:], in0=ar2[:, 1:2],
                                    scalar1=self.inv_n)
        musq = con.tile([C, 1], F32, name=f"mq2_{s}")
        nc.vector.tensor_mul(out=musq[:], in0=mu[:], in1=mu[:])
        nc.vector.tensor_sub(out=var[:], in0=var[:], in1=musq[:])
        std = con.tile([C, 1], F32, name=f"sd2_{s}")
        nc.scalar.activation(out=std[:], in_=var[:], func=AF.Sqrt,
                             bias=self.p["epsb"][:])
        inv = con.tile([C, 1], F32, name=f"iv2_{s}")
        nc.vector.reciprocal(out=inv[:], in_=std[:])
        self.w3ts = con.tile([C, M], F16, name=f"w3s_{s}")
        nc.vector.tensor_scalar_mul(out=self.w3ts[:], in0=self.p["w3gt"][:],
                                    scalar1=inv[:])
        s2 = con.tile([C, 1], F32, name=f"s2_{s}")
        nc.vector.tensor_mul(out=s2[:], in0=inv[:], in1=mu[:])
        self.ccol = con.tile([C, 1], F32, name=f"cc_{s}")
        nc.vector.tensor_mul(out=self.ccol[:], in0=s2[:], in1=cols[:, 6:7])
        nc.vector.tensor_sub(out=self.ccol[:], in0=cols[:, 5:6],
                             in1=self.ccol[:])

    # ---------------- phase C ----------------
    def stage_c(self, psc):
        nc, s = self.nc, self.s
        ost = self.p["ost"]
        out = self.io["out"][s]
        for b0 in range(0, self.NT, 2):
            nb = min(2, self.NT - b0)
            pc = psc.tile([M, 2, 512], F32, tag="ptc")
            for j in range(nb):
                tt = b0 + j
                nc.tensor.matmul(out=pc[:, j, :], lhsT=self.w3ts[:],
                                 rhs=self.opre[:, 512 * tt:512 * (tt + 1)],
                                 start=True, stop=True)
            o = ost.tile([C, 2, 512], F32, tag="o")
            if (b0 // 2) % 2 == 0:
                nc.vector.tensor_scalar(out=o[:, 0:nb, :],
                                        in0=pc[0:C, 0:nb, :],
                                        scalar1=self.ccol[:], scalar2=None,
                                        op0=ALU.add)
            else:
                nc.scalar.activation(out=o[:, 0:nb, :], in_=pc[0:C, 0:nb, :],
                                     func=AF.Identity, bias=self.ccol[:])
            nc.sync.dma_start(
                out=out[:, 2 * b0:2 * b0 + 2 * nb, :].rearrange(
                    "p (n r) w -> p n r w", r=2),
                in_=o[:, 0:nb, :].rearrange("p n (r w) -> p n r w", w=W))


def _emit(nc, tc, ctx, RH, groups, io):
    pools = {
        "consts": ctx.enter_context(tc.tile_pool(name="consts", bufs=1)),
        "big": ctx.enter_context(tc.tile_pool(name="big", bufs=1)),
        "xin": ctx.enter_context(tc.tile_pool(name="xin", bufs=4)),
        "gst": ctx.enter_context(tc.tile_pool(name="gst", bufs=3)),
        "tmp": ctx.enter_context(tc.tile_pool(name="tmp", bufs=4)),
        "ost": ctx.enter_context(tc.tile_pool(name="ost", bufs=4)),
        "dram": ctx.enter_context(tc.tile_pool(name="dram", bufs=1,
                                               space="DRAM")),
    }
    con = pools["consts"]
    w1t = con.tile([C, M], F16)
    nc.sync.dma_start(out=w1t[:], in_=io["w1t"][:])
    wbm = con.tile([C + 1, 6 * M], F16)
    nc.sync.dma_start(out=wbm[:], in_=io["wbm"][:])
    w3gt = con.tile([C, M], F32)
    nc.sync.dma_start(out=w3gt[:], in_=io["w3gt"][:])
    cols = con.tile([C, 7], F32)
    nc.sync.dma_start(out=cols[:], in_=io["cols"][:])
    em = con.tile([C, 2 * S], F32)
    nc.gpsimd.dma_start(out=em[:], in_=_bcast(io["em"][:], C))
    ones96 = con.tile([C, 1], F32)
    nc.vector.memset(ones96[:], 1.0)
    epsb = con.tile([C, 1], F32)
    nc.vector.memset(epsb[:], EPS)
    zcol = con.tile([C, 1], F32)
    nc.vector.memset(zcol[:], 0.0)
    pools.update(w1t=w1t, wbm=wbm, w3gt=w3gt, cols=cols, em=em,
                 ones96=ones96, epsb=epsb, zcol=zcol)

    # warm up the collectives firmware path (result unused)
    dw_i = pools["dram"].tile([1, 2], F32)
    dw_o = pools["dram"].tile([1, 2], F32)
    warm = con.tile([1, 2], F32)
    nc.vector.memset(warm[:], 0.0)
    nc.sync.dma_start(out=dw_i[:], in_=warm[:])
    nc.gpsimd.collective_compute("AllReduce", ALU.add, replica_groups=groups,
                                 ins=[dw_i.opt()], outs=[dw_o.opt()])

    streams = [_Stream(nc, tc, pools, RH, groups, io, s) for s in range(S)]

    with tc.tile_pool(name="psa", bufs=4, space="PSUM") as psa:
        for st in streams:
            st.stage_a(psa)
    with tc.tile_pool(name="psb", bufs=2, space="PSUM") as psb:
        for st in streams:
            st.post_ar1()
            st.stage_b(psb)
    with tc.tile_pool(name="psc", bufs=4, space="PSUM") as psc:
        for st in streams:
            st.post_ar2()
            st.stage_c(psc)


def build_program(rows_slab=ROWS_SLAB, n_cores=N_CORES, n_samples=B,
                  n_streams=S):
    import contextlib
    cps = n_cores * n_streams // n_samples      # cores per sample
    groups = [list(range(a * cps, (a + 1) * cps))
              for a in range(n_cores // cps)]
    RF = rows_slab + 2
    nc = bacc.Bacc("TRN2", target_bir_lowering=False, debug=False,
                   enable_asserts=False, num_devices=n_cores)
    io = {
        "xs": nc.dram_tensor("xs", [n_streams, C, RF, W], F16,
                             kind="ExternalInput").ap(),
        "em": nc.dram_tensor("em", [1, 2 * n_streams], F32,
                             kind="ExternalInput").ap(),
        "w1t": nc.dram_tensor("w1t", [C, M], F16, kind="ExternalInput").ap(),
        "wbm": nc.dram_tensor("wbm", [C + 1, 6 * M], F16,
                              kind="ExternalInput").ap(),
        "w3gt": nc.dram_tensor("w3gt", [C, M], F32, kind="ExternalInput").ap(),
        "cols": nc.dram_tensor("cols", [C, 7], F32, kind="ExternalInput").ap(),
        "onesrow": nc.dram_tensor("onesrow", [1, WP], F16,
                                  kind="ExternalInput").ap(),
        "out": nc.dram_tensor("out", [n_streams, C, rows_slab, W], F32,
                              kind="ExternalOutput").ap(),
    }
    with tile.TileContext(nc) as tc:
        with contextlib.ExitStack() as ctx:
            _emit(nc, tc, ctx, rows_slab, groups, io)
    nc.compile()
    return nc


def host_inputs(x, w1, b1, w21, b21, w22, b22, w3, b3,
                gn1_w, gn1_b, gn2_w, gn2_b,
                rows_slab=ROWS_SLAB, n_cores=N_CORES, n_streams=S):
    x = np.asarray(x, np.float32)
    nb_, _, hh, _ = x.shape
    cps = n_cores * n_streams // nb_
    w1 = np.asarray(w1, np.float32)
    w21 = np.asarray(w21, np.float32)
    w22 = np.asarray(w22, np.float32)
    w3 = np.asarray(w3, np.float32)

    w1t = np.zeros((C, M), np.float16)
    w1t[:, 0:C] = w1.T
    wbm = np.zeros((C + 1, 6 * M), np.float16)
    biases = (np.asarray(b21, np.float32), np.asarray(b22, np.float32))
    for wi, wmat in enumerate((w21, w22)):
        wt = np.ascontiguousarray(wmat.T).astype(np.float16)
        for j in range(3):
            blk = np.zeros((C + 1, M), np.float16)
            blk[32 * j:32 * j + 32, 0:C] = wt[32 * j:32 * j + 32, :]
            if j == 0:
                blk[C, 0:C] = biases[wi].astype(np.float16)
            wbm[:, (3 * wi + j) * M:(3 * wi + j + 1) * M] = blk
    w3gt = np.zeros((C, M), np.float32)
    w3gt[:, 0:C] = (w3 * np.asarray(gn2_w)[None, :]).T
    shared = {
        "w1t": w1t,
        "wbm": wbm,
        "onesrow": np.ones((1, WP), np.float16),
        "w3gt": w3gt,
        "cols": np.ascontiguousarray(np.stack(
            [np.asarray(b1, np.float32), np.asarray(gn1_w, np.float32),
             np.asarray(gn1_b, np.float32), np.asarray(b21, np.float32),
             np.asarray(b22, np.float32),
             (np.asarray(b3) + w3 @ np.asarray(gn2_b)).astype(np.float32),
             (w3 * np.asarray(gn2_w)[None, :]).sum(1).astype(np.float32)],
            axis=1)),
    }
    x16 = x.astype(np.float16)
    in_maps = []
    for k in range(n_cores):
        xs = np.zeros((n_streams, C, rows_slab + 2, W), np.float16)
        em = np.zeros((1, 2 * n_streams), np.float32)
        for s in range(n_streams):
            bidx = s * (nb_ // n_streams) + k // cps
            q = k % cps
            h0 = q * rows_slab
            lo, hi = h0 - 1, h0 + rows_slab + 1
            slo, shi = max(lo, 0), min(hi, hh)
            xs[s, :, slo - lo:slo - lo + (shi - slo), :] = \
                x16[bidx, :, slo:shi, :]
            em[0, 2 * s] = 1.0 if lo >= 0 else 0.0
            em[0, 2 * s + 1] = 1.0 if hi <= hh else 0.0
        in_maps.append({"xs": xs, "em": em, **shared})
    return in_maps


def gather_output(results, rows_slab=ROWS_SLAB, n_cores=N_CORES,
                  n_streams=S, n_samples=B, hh=H):
    cps = n_cores * n_streams // n_samples
    out = np.empty((n_samples, C, hh, W), np.float32)
    for k in range(n_cores):
        for s in range(n_streams):
            bidx = s * (n_samples // n_streams) + k // cps
            q = k % cps
            out[bidx, :, q * rows_slab:(q + 1) * rows_slab, :] = \
                results[k]["out"][s]
    return out


_PROGRAM = None


def kernel(x, w1, b1, w21, b21, w22, b22, w3, b3, gn1_w, gn1_b, gn2_w, gn2_b):
    global _PROGRAM
    from concourse.bass_utils import run_bass_kernel_spmd
    from concourse.bass_interp import get_hw_module
    if _PROGRAM is None:
        nc = build_program()
        nc.m = get_hw_module(nc.m)
        _PROGRAM = nc
    nc = _PROGRAM
    in_maps = host_inputs(x, w1, b1, w21, b21, w22, b22, w3, b3,
                          gn1_w, gn1_b, gn2_w, gn2_b)
    res = run_bass_kernel_spmd(nc, in_maps, core_ids=list(range(N_CORES)))
    return gather_output(res.results)
